# revision 1
# baseline (speedup 1.0000x reference)
"""DFlashDraftModel Trainium2 kernel — 8 NeuronCores, DP2 x TP4.

Sharding: cores 0-3 handle batches 0-1, cores 4-7 handle batches 2-3 (DP groups).
Within each group of 4 (TP): fc output-dim sharded (640/core) + AllGather;
q heads 5/core, kv heads 1/core, o input-sharded, gate/up 1728/core,
down input-sharded; AllReduce after o_proj and down_proj within the group.

All matmuls bf16 with fp32 PSUM accumulation. RMS-norm scales are commuted
through matmuls (applied to outputs); norm weight vectors are folded into
weights / rope tables / host post-scale.
"""
import sys

if '/opt/trn_rl_repo' not in sys.path:
    sys.path.insert(0, '/opt/trn_rl_repo')

import numpy as np
import ml_dtypes

import concourse.bass as bass
import concourse.tile as tile
from concourse import bacc, mybir
from concourse import bass_utils
from concourse.masks import make_identity

BF16 = mybir.dt.bfloat16
F32 = mybir.dt.float32
AF = mybir.ActivationFunctionType
OP = mybir.AluOpType

L = 5; B = 4; Q = 16; CTX = 2048; S = CTX + Q
H = 2560; I = 6912; NH = 20; NKV = 4; HD = 128; NF = 5
EPS = 1e-6; THETA = 1000000.0

NCORES = 8
TP = 4                      # tensor-parallel group size
GROUPS = [[0, 1, 2, 3], [4, 5, 6, 7]]
TOK = 2 * CTX               # ctx tokens per core (2 batches)
NTC = TOK // 128            # 32 context token chunks per core
FIN = NF * H                # 12800
KF = FIN // 128             # 100 fc contraction chunks
HSH = H // TP               # 640  fc output shard per core
ISH = I // TP               # 1728 gate/up shard
QH = NH // TP               # 5 q heads per core
DTOK = 2 * Q                # 32 decoder tokens per core
NKC = 20                    # H/128 contraction chunks
NFC = 8                     # fc token chunks (512 each)
FCN = 512                   # fc token chunk width
GU2 = 2 * ISH               # 3456
# gate/up interleaved column chunks: [g0,u0,g1,u1,g2,u2,(g3|u3)]
GU_CH = [512, 512, 512, 512, 512, 512, 384]
DKC = 14                    # down contraction chunks: 13*128 + 64
DK_LAST = ISH - 13 * 128    # 64


def _build():
    nc = bacc.Bacc("TRN2", target_bir_lowering=False, debug=False,
                   enable_asserts=False, num_devices=NCORES)

    # ---- I/O ----
    xt_d = nc.dram_tensor("xt", [FIN, TOK], BF16, kind="ExternalInput").ap()
    fcw_d = nc.dram_tensor("fcw", [FIN, HSH], BF16, kind="ExternalInput").ap()
    kvw_d = nc.dram_tensor("kvw", [L, H, 256], BF16, kind="ExternalInput").ap()
    qkvw_d = nc.dram_tensor("qkvw", [L, H, 896], BF16, kind="ExternalInput").ap()
    ow_d = nc.dram_tensor("ow", [L, HSH, H], BF16, kind="ExternalInput").ap()
    guw_d = nc.dram_tensor("guw", [L, H, GU2], BF16, kind="ExternalInput").ap()
    dw_d = nc.dram_tensor("dw", [L, ISH, H], BF16, kind="ExternalInput").ap()
    ckc_d = nc.dram_tensor("ckc", [L, 2, CTX, HD], BF16, kind="ExternalInput").ap()
    skc_d = nc.dram_tensor("skc", [L, 2, CTX, HD], BF16, kind="ExternalInput").ap()
    ckq_d = nc.dram_tensor("ckq", [L, DTOK, HD], BF16, kind="ExternalInput").ap()
    skq_d = nc.dram_tensor("skq", [L, DTOK, HD], BF16, kind="ExternalInput").ap()
    ckd_d = nc.dram_tensor("ckd", [L, DTOK, HD], BF16, kind="ExternalInput").ap()
    skd_d = nc.dram_tensor("skd", [L, DTOK, HD], BF16, kind="ExternalInput").ap()
    x0_d = nc.dram_tensor("x0", [DTOK, H], F32, kind="ExternalInput").ap()
    y_d = nc.dram_tensor("y", [DTOK, H], F32, kind="ExternalOutput").ap()

    with tile.TileContext(nc) as tc:
        _emit(nc, tc, xt_d, fcw_d, kvw_d, qkvw_d, ow_d, guw_d, dw_d,
              ckc_d, skc_d, ckq_d, skq_d, ckd_d, skd_d, x0_d, y_d)
    nc.compile()
    return nc


def _rsqrt(nc, pool, ss, n_mean, ptok, name):
    """rsqrt(ss/n_mean + EPS) for per-partition scalars ss [ptok,1] f32."""
    m = pool.tile([ptok, 1], F32, name=f"m_{name}")
    nc.vector.tensor_scalar(out=m[:], in0=ss[:], scalar1=1.0 / n_mean,
                            scalar2=EPS, op0=OP.mult, op1=OP.add)
    r = pool.tile([ptok, 1], F32, name=f"r_{name}")
    nc.vector.reciprocal(r[:], m[:])
    o = pool.tile([ptok, 1], F32, name=f"o_{name}")
    nc.scalar.sqrt(o[:], r[:])
    return o


def _rope(nc, pool, src, ck, sk, ptok, name):
    """out = src*ck + rotate_half_pair(src)*sk  (sign folded into sk).
    src/ck/sk: [ptok, 128] bf16 APs. Returns bf16 tile."""
    t1 = pool.tile([ptok, HD], BF16, name=f"t1_{name}")
    nc.vector.tensor_tensor(out=t1[:], in0=src, in1=ck, op=OP.mult)
    t2 = pool.tile([ptok, HD], BF16, name=f"t2_{name}")
    nc.vector.tensor_tensor(out=t2[:, 0:64], in0=src[:, 64:128],
                            in1=sk[:, 0:64], op=OP.mult)
    nc.vector.tensor_tensor(out=t2[:, 64:128], in0=src[:, 0:64],
                            in1=sk[:, 64:128], op=OP.mult)
    t3 = pool.tile([ptok, HD], BF16, name=f"t3_{name}")
    nc.vector.tensor_tensor(out=t3[:], in0=t1[:], in1=t2[:], op=OP.add)
    return t3


def _emit(nc, tc, xt_d, fcw_d, kvw_d, qkvw_d, ow_d, guw_d, dw_d,
          ckc_d, skc_d, ckq_d, skq_d, ckd_d, skd_d, x0_d, y_d):
    with tc.tile_pool(name="glob", bufs=1) as glob, \
         tc.tile_pool(name="dramg", bufs=1, space="DRAM") as dram:
        # ---------------- long-lived tiles ----------------
        ident = glob.tile([128, 128], BF16, name="ident")
        make_identity(nc, ident[:])
        # shifted identity: ident_sh[16+j, j] = 1 (selects rows 16:32)
        ident_sh = glob.tile([128, 128], BF16, name="ident_sh")
        nc.gpsimd.memset(ident_sh[:], 0.0)
        nc.gpsimd.affine_select(out=ident_sh[:], in_=ident_sh[:],
                                compare_op=OP.not_equal, fill=1.0, base=-16,
                                pattern=[[-1, 128]], channel_multiplier=1)
        ones = glob.tile([128, 1], BF16, name="ones")
        nc.vector.memset(ones[:], 1.0)
        i1f = glob.tile([1, 1], F32, name="i1f")
        nc.vector.memset(i1f[:], 1.0)
        x_sb = glob.tile([DTOK, H], F32, name="x_sb")
        nc.sync.dma_start(out=x_sb[:], in_=x0_d[:])
        invT = glob.tile([128, NTC], F32, name="invT")

        th_n = [dram.tile([H, FCN], BF16, name=f"th{n}") for n in range(NFC)]
        ssq_in = dram.tile([1, TOK], F32, name="ssq_in")
        ssq_out = dram.tile([1, TOK], F32, name="ssq_out")

        # ---------------- Phase 1: fc ----------------
        with tc.tile_pool(name="fcw_pool", bufs=1) as fcwp, \
             tc.tile_pool(name="xtp", bufs=6) as xtp, \
             tc.tile_pool(name="fcps", bufs=6, space="PSUM") as fcps, \
             tc.tile_pool(name="ssqps", bufs=2, space="PSUM") as ssqps, \
             tc.tile_pool(name="fcev", bufs=4) as fcev, \
             tc.tile_pool(name="ssqrp", bufs=1) as ssqrp, \
             tc.tile_pool(name="agp", bufs=2, space="DRAM") as agp:
            fcw_sb = fcwp.tile([128, KF, HSH], BF16)
            ssq_row = ssqrp.tile([1, TOK], F32, name="ssq_row")
            fcw_r = fcw_d.rearrange("(kk p) m -> p kk m", p=128)
            for qq in range(4):
                nc.sync.dma_start(
                    out=fcw_sb[:, 25 * qq:25 * (qq + 1), :],
                    in_=fcw_r[:, 25 * qq:25 * (qq + 1), :])
            for n in range(NFC):
                psF = [fcps.tile([128, FCN], F32, name="psF", tag="psF")
                       for _ in range(5)]
                for k in range(KF):
                    xt_t = xtp.tile([128, FCN], BF16, name="xt_t", tag="xt_t")
                    nc.sync.dma_start(
                        out=xt_t[:],
                        in_=xt_d[128 * k:128 * (k + 1), FCN * n:FCN * (n + 1)])
                    for m in range(5):
                        nc.tensor.matmul(
                            psF[m][:], fcw_sb[:, k, 128 * m:128 * (m + 1)],
                            xt_t[:], start=(k == 0), stop=(k == KF - 1))
                ag_in = agp.tile([HSH, FCN], BF16, name="ag_in", tag="ag_in")
                ssq_ps = ssqps.tile([1, FCN], F32, name="ssq_ps", tag="ssq_ps")
                for m in range(5):
                    th_bf = fcev.tile([128, FCN], BF16, name="th_bf", tag="th_bf")
                    nc.vector.tensor_copy(th_bf[:], psF[m][:])
                    nc.sync.dma_start(out=ag_in[128 * m:128 * (m + 1), :],
                                      in_=th_bf[:])
                    th2 = fcev.tile([128, FCN], BF16, name="th2", tag="th2")
                    nc.scalar.square(th2[:], psF[m][:])
                    nc.tensor.matmul(ssq_ps[:], ones[:], th2[:],
                                     start=(m == 0), stop=(m == 4))
                nc.vector.tensor_copy(ssq_row[:, FCN * n:FCN * (n + 1)], ssq_ps[:])
                nc.gpsimd.collective_compute(
                    "AllGather", OP.bypass, replica_groups=GROUPS,
                    ins=[ag_in[:]], outs=[th_n[n][:]])
            # sum-of-squares all-reduce + inv_rms in [128, NTC] layout
            nc.sync.dma_start(out=ssq_in[:], in_=ssq_row[:])
            nc.gpsimd.collective_compute(
                "AllReduce", OP.add, replica_groups=GROUPS,
                ins=[ssq_in[:]], outs=[ssq_out[:]])

        # inv_rms of th in [128, NTC] per-partition layout
        with tc.tile_pool(name="ivps", bufs=2, space="PSUM") as ivps, \
             tc.tile_pool(name="ivtmp", bufs=2) as ivtmp:
            sro = ivtmp.tile([1, TOK], F32, name="sro")
            nc.sync.dma_start(out=sro[:], in_=ssq_out[:])
            ssT = ivtmp.tile([128, NTC], F32, name="ssT")
            for j in range(NTC):
                tp = ivps.tile([128, 1], F32, name="tp_iv", tag="tp_iv")
                nc.tensor.matmul(tp[:], sro[:, 128 * j:128 * (j + 1)],
                                 i1f[:], start=True, stop=True)
                nc.vector.tensor_copy(ssT[:, j:j + 1], tp[:])
            m = ivtmp.tile([128, NTC], F32, name="m_iv")
            nc.vector.tensor_scalar(out=m[:], in0=ssT[:], scalar1=1.0 / H,
                                    scalar2=EPS, op0=OP.mult, op1=OP.add)
            r = ivtmp.tile([128, NTC], F32, name="r_iv")
            nc.vector.reciprocal(r[:], m[:])
            nc.scalar.sqrt(invT[:], r[:])

        # ---------------- Phases 2+3: per-layer ----------------
        with tc.tile_pool(name="kvwp", bufs=2) as kvwp, \
             tc.tile_pool(name="ktc", bufs=3) as ktcp, \
             tc.tile_pool(name="vcp", bufs=3) as vcp, \
             tc.tile_pool(name="tht", bufs=2) as thtp, \
             tc.tile_pool(name="kvps", bufs=2, space="PSUM") as kvps, \
             tc.tile_pool(name="ktps", bufs=2, space="PSUM") as ktps, \
             tc.tile_pool(name="kvpost", bufs=3) as kvpost, \
             tc.tile_pool(name="sc1", bufs=2) as sc1, \
             tc.tile_pool(name="arb", bufs=1) as arb, \
             tc.tile_pool(name="decw", bufs=3) as decw, \
             tc.tile_pool(name="dps", bufs=4, space="PSUM") as dps, \
             tc.tile_pool(name="attp", bufs=3) as attp, \
             tc.tile_pool(name="decx", bufs=2) as decx, \
             tc.tile_pool(name="ardr", bufs=2, space="DRAM") as ardr:
            for l in range(L):
                # ===== KV build over context =====
                kvw_sb = kvwp.tile([128, NKC, 256], BF16, name="kvw_sb", tag="kvw")
                nc.sync.dma_start(
                    out=kvw_sb[:],
                    in_=kvw_d[l].rearrange("(kk p) c -> p kk c", p=128))
                ckc_r = ckc_d[l].rearrange("b (c p) d -> p (b c) d", p=128)
                skc_r = skc_d[l].rearrange("b (c p) d -> p (b c) d", p=128)
                KT_c = ktcp.tile([128, 2, S], BF16, name="KT_c", tag="KT")
                V_c = vcp.tile([128, 2, 17, HD], BF16, name="V_c", tag="V")
                for t_ in range(NTC):
                    b, c = t_ // 16, t_ % 16
                    thT_t = thtp.tile([128, NKC, 128], BF16, name="thT_t",
                                      tag="thT")
                    nc.sync.dma_start(
                        out=thT_t[:],
                        in_=th_n[t_ // 4].rearrange(
                            "(kk p) (cc w) -> p kk cc w", p=128, w=128)
                            [:, :, t_ % 4, :])
                    kv_ps = kvps.tile([128, 256], F32, name="kv_ps", tag="kv_ps")
                    for k in range(NKC):
                        nc.tensor.matmul(kv_ps[:], thT_t[:, k, :], kvw_sb[:, k, :],
                                         start=(k == 0), stop=(k == NKC - 1))
                    iv = invT[:, t_:t_ + 1]
                    kvs = kvpost.tile([128, 256], BF16, name="kvs", tag="kvs")
                    nc.vector.tensor_scalar_mul(kvs[:], kv_ps[:], iv)
                    nc.vector.tensor_copy(V_c[:, b, c, :], kvs[:, 128:256])
                    k2t = kvpost.tile([128, HD], BF16, name="k2t", tag="k2t")
                    ks = kvpost.tile([128, 1], F32, name="ks", tag="ks")
                    nc.scalar.activation(k2t[:], kvs[:, 0:128], AF.Square,
                                         accum_out=ks[:])
                    rs = _rsqrt(nc, kvpost, ks, HD, 128, "kh")
                    khn = kvpost.tile([128, HD], BF16, name="khn", tag="khn")
                    nc.vector.tensor_scalar_mul(khn[:], kvs[:, 0:128], rs[:])
                    ck_t = kvpost.tile([128, HD], BF16, name="ck_t",
                                       tag="ck_t")
                    nc.sync.dma_start(out=ck_t[:], in_=ckc_r[:, t_, :])
                    sk_t = kvpost.tile([128, HD], BF16, name="sk_t",
                                       tag="sk_t")
                    nc.sync.dma_start(out=sk_t[:], in_=skc_r[:, t_, :])
                    kr = _rope(nc, kvpost, khn[:], ck_t[:],
                               sk_t[:], 128, "kc")
                    kt_ps = ktps.tile([128, 128], BF16, name="kt_ps", tag="kt_ps")
                    nc.tensor.transpose(kt_ps[:], kr[:], ident[:])
                    nc.vector.tensor_copy(KT_c[:, b, 128 * c:128 * (c + 1)],
                                          kt_ps[:])

                # ===== decoder layer =====
                # inv_rms1 of x
                xsq = sc1.tile([DTOK, H], BF16, name="xsq", tag="xsq")
                ssx = sc1.tile([DTOK, 1], F32, name="ssx", tag="ssx")
                nc.scalar.activation(xsq[:], x_sb[:], AF.Square, accum_out=ssx[:])
                inv1 = _rsqrt(nc, sc1, ssx, H, DTOK, "in1")
                # xT (raw x, bf16)
                xb = sc1.tile([DTOK, H], BF16, name="xb", tag="xb")
                nc.vector.tensor_copy(xb[:], x_sb[:])
                xT = decx.tile([128, NKC, DTOK], BF16, name="xT", tag="xT")
                for k in range(NKC):
                    tp = ktps.tile([128, DTOK], BF16, name="tp_x", tag="kt_ps")
                    nc.tensor.transpose(tp[:], xb[:, 128 * k:128 * (k + 1)],
                                        ident[:DTOK, :DTOK])
                    nc.vector.tensor_copy(xT[:, k, :], tp[:])
                # qkv projection
                q_ps = dps.tile([DTOK, 512], F32, name="q_ps", tag="acc")
                q_ps2 = dps.tile([DTOK, 384], F32, name="q_ps2", tag="acc")
                for k in range(NKC):
                    wt = decw.tile([128, 896], BF16, name="qkvwt", tag="qkvwt")
                    nc.sync.dma_start(out=wt[:],
                                      in_=qkvw_d[l, 128 * k:128 * (k + 1), :])
                    nc.tensor.matmul(q_ps[:], xT[:, k, :], wt[:, 0:512],
                                     start=(k == 0), stop=(k == NKC - 1))
                    nc.tensor.matmul(q_ps2[:], xT[:, k, :], wt[:, 512:896],
                                     start=(k == 0), stop=(k == NKC - 1))
                # q: scale by inv1, per-head rms, rope
                q1 = sc1.tile([DTOK, QH * HD], BF16, name="q1", tag="q1")
                nc.vector.tensor_scalar_mul(q1[:, 0:512], q_ps[:], inv1[:])
                nc.vector.tensor_scalar_mul(q1[:, 512:640], q_ps2[:, 0:128],
                                            inv1[:])
                ckq_sb = sc1.tile([DTOK, HD], BF16, name="ckq_sb", tag="ckq")
                skq_sb = sc1.tile([DTOK, HD], BF16, name="skq_sb", tag="skq")
                nc.sync.dma_start(out=ckq_sb[:], in_=ckq_d[l])
                nc.sync.dma_start(out=skq_sb[:], in_=skq_d[l])
                qr = sc1.tile([DTOK, QH * HD], BF16, name="qr", tag="qr")
                for h in range(QH):
                    hs = slice(128 * h, 128 * (h + 1))
                    q2h = sc1.tile([DTOK, HD], BF16, name="q2h", tag="q2h")
                    qsh = sc1.tile([DTOK, 1], F32, name="qsh", tag="qsh")
                    nc.scalar.activation(q2h[:], q1[:, hs], AF.Square,
                                         accum_out=qsh[:])
                    rqh = _rsqrt(nc, sc1, qsh, HD, DTOK, f"qh{h}")
                    qhn = sc1.tile([DTOK, HD], BF16, name="qhn", tag="qhn")
                    nc.vector.tensor_scalar_mul(qhn[:], q1[:, hs], rqh[:])
                    qrh = _rope(nc, sc1, qhn[:], ckq_sb[:], skq_sb[:], DTOK, "q")
                    nc.vector.tensor_copy(qr[:, hs], qrh[:])
                # qT per batch
                qT = [sc1.tile([128, QH * Q], BF16, name=f"qT{b}", tag=f"qT{b}")
                      for b in range(2)]
                for b in range(2):
                    sel = ident[:DTOK, 0:Q] if b == 0 else ident_sh[:DTOK, 0:Q]
                    for h in range(QH):
                        tqp = ktps.tile([128, Q], F32, name="tqp", tag="kt_ps")
                        nc.tensor.matmul(tqp[:], qr[:, 128 * h:128 * (h + 1)],
                                         sel, start=True, stop=True)
                        nc.vector.tensor_copy(qT[b][:, Q * h:Q * (h + 1)], tqp[:])
                # kd/vd
                kvd = sc1.tile([DTOK, 256], BF16, name="kvd", tag="kvd")
                nc.vector.tensor_scalar_mul(kvd[:], q_ps2[:, 128:384], inv1[:])
                k2d = sc1.tile([DTOK, HD], BF16, name="k2d", tag="k2d")
                ksd = sc1.tile([DTOK, 1], F32, name="ksd", tag="ksd")
                nc.scalar.activation(k2d[:], kvd[:, 0:128], AF.Square,
                                     accum_out=ksd[:])
                rsd = _rsqrt(nc, sc1, ksd, HD, DTOK, "kd")
                khd = sc1.tile([DTOK, HD], BF16, name="khd", tag="khd")
                nc.vector.tensor_scalar_mul(khd[:], kvd[:, 0:128], rsd[:])
                ckd_sb = sc1.tile([DTOK, HD], BF16, name="ckd_sb", tag="ckd")
                skd_sb = sc1.tile([DTOK, HD], BF16, name="skd_sb", tag="skd")
                nc.sync.dma_start(out=ckd_sb[:], in_=ckd_d[l])
                nc.sync.dma_start(out=skd_sb[:], in_=skd_d[l])
                krd = _rope(nc, sc1, khd[:], ckd_sb[:], skd_sb[:], DTOK, "kd")
                for b in range(2):
                    sel = ident[:DTOK, 0:Q] if b == 0 else ident_sh[:DTOK, 0:Q]
                    kdp = ktps.tile([128, Q], F32, name="kdp", tag="kt_ps")
                    nc.tensor.matmul(kdp[:], krd[:], sel, start=True, stop=True)
                    nc.vector.tensor_copy(KT_c[:, b, CTX:S], kdp[:])
                    # vd: partition shift via sbuf->sbuf dma
                    nc.sync.dma_start(out=V_c[0:Q, b, 16, :],
                                      in_=kvd[Q * b:Q * (b + 1), 128:256])
                # attention per batch
                attn_cat = sc1.tile([128, 2 * QH * Q], BF16, name="attn_cat",
                                    tag="attn_cat")
                for b in range(2):
                    rs_ps = dps.tile([1, QH * Q], F32, name="rs_ps", tag="acc")
                    ao_ps = dps.tile([QH * Q, HD], F32, name="ao_ps", tag="acc")
                    for s_ in range(17):
                        klen = 128 if s_ < 16 else Q
                        sc_ps = dps.tile([128, QH * Q], F32, name="sc_ps", tag="acc")
                        nc.tensor.matmul(
                            sc_ps[:klen, :],
                            KT_c[:, b, 128 * s_:128 * s_ + klen], qT[b][:],
                            start=True, stop=True)
                        at = attp.tile([128, QH * Q], BF16, name="at", tag="at")
                        nc.scalar.activation(at[:klen, :], sc_ps[:klen, :], AF.Exp)
                        nc.tensor.matmul(rs_ps[:], ones[:klen, :], at[:klen, :],
                                         start=(s_ == 0), stop=(s_ == 16))
                        nc.tensor.matmul(ao_ps[:], at[:klen, :],
                                         V_c[:klen, b, s_, :],
                                         start=(s_ == 0), stop=(s_ == 16))
                    rs_sb = sc1.tile([1, QH * Q], F32, name="rs_sb", tag="rs_sb")
                    nc.vector.tensor_copy(rs_sb[:], rs_ps[:])
                    rsT_ps = ktps.tile([QH * Q, 1], F32, name="rsT_ps",
                                       tag="kt_ps")
                    nc.tensor.matmul(rsT_ps[:], rs_sb[:], i1f[:],
                                     start=True, stop=True)
                    rinv = sc1.tile([QH * Q, 1], F32, name="rinv", tag="rinv")
                    nc.vector.reciprocal(rinv[:], rsT_ps[:])
                    aob = sc1.tile([QH * Q, HD], BF16, name="aob", tag="aob")
                    nc.vector.tensor_scalar_mul(aob[:], ao_ps[:], rinv[:])
                    aoT_ps = ktps.tile([128, QH * Q], BF16, name="aoT_ps",
                                       tag="kt_ps")
                    nc.tensor.transpose(aoT_ps[:], aob[:], ident[:QH * Q, :QH * Q])
                    nc.vector.tensor_copy(
                        attn_cat.rearrange("p (h bt) -> p h bt", bt=2 * Q)
                            [:, :, Q * b:Q * (b + 1)],
                        aoT_ps.rearrange("p (h t) -> p h t", t=Q)[:])
                # o projection
                oar = arb.tile([DTOK, H], F32, name="oar", tag="oar")
                for nns in [(0, 3), (3, 5)]:
                    o_ps = {nn: dps.tile([DTOK, 512], F32, name="o_ps", tag="acc")
                            for nn in range(*nns)}
                    for h in range(QH):
                        for nn in range(*nns):
                            owt = decw.tile([128, 512], BF16, name="owt",
                                            tag="owt")
                            nc.sync.dma_start(
                                out=owt[:],
                                in_=ow_d[l, 128 * h:128 * (h + 1),
                                         512 * nn:512 * (nn + 1)])
                            nc.tensor.matmul(
                                o_ps[nn][:], attn_cat[:, 32 * h:32 * (h + 1)],
                                owt[:], start=(h == 0), stop=(h == QH - 1))
                    for nn in range(*nns):
                        nc.vector.tensor_copy(oar[:, 512 * nn:512 * (nn + 1)],
                                              o_ps[nn][:])
                oin = ardr.tile([DTOK, H], F32, name="oin", tag="oin")
                oout = ardr.tile([DTOK, H], F32, name="oout", tag="oout")
                nc.sync.dma_start(out=oin[:], in_=oar[:])
                nc.gpsimd.collective_compute("AllReduce", OP.add,
                                             replica_groups=GROUPS,
                                             ins=[oin[:]], outs=[oout[:]])
                oas = arb.tile([DTOK, H], F32, name="oas", tag="oas")
                nc.sync.dma_start(out=oas[:], in_=oout[:])
                nc.vector.tensor_tensor(out=x_sb[:], in0=x_sb[:], in1=oas[:],
                                        op=OP.add)
                # inv_rms2 + x2T
                xsq2 = sc1.tile([DTOK, H], BF16, name="xsq2", tag="xsq")
                ssx2 = sc1.tile([DTOK, 1], F32, name="ssx2", tag="ssx")
                nc.scalar.activation(xsq2[:], x_sb[:], AF.Square,
                                     accum_out=ssx2[:])
                inv2 = _rsqrt(nc, sc1, ssx2, H, DTOK, "in2")
                xb2 = sc1.tile([DTOK, H], BF16, name="xb2", tag="xb")
                nc.vector.tensor_copy(xb2[:], x_sb[:])
                x2T = decx.tile([128, NKC, DTOK], BF16, name="x2T", tag="x2T")
                for k in range(NKC):
                    tp2 = ktps.tile([128, DTOK], BF16, name="tp_x2", tag="kt_ps")
                    nc.tensor.transpose(tp2[:], xb2[:, 128 * k:128 * (k + 1)],
                                        ident[:DTOK, :DTOK])
                    nc.vector.tensor_copy(x2T[:, k, :], tp2[:])
                # gate/up
                h_sb = sc1.tile([DTOK, ISH], BF16, name="h_sb", tag="h_sb")
                gu_off = [0, 512, 1024, 1536, 2048, 2560, 3072]
                for jlo, jhi in [(0, 4), (4, 7)]:
                    clo, chi = gu_off[jlo], gu_off[jhi - 1] + GU_CH[jhi - 1]
                    gu_ps = {j: dps.tile([DTOK, GU_CH[j]], F32, name=f"gu{j}",
                                         tag="acc") for j in range(jlo, jhi)}
                    for k in range(NKC):
                        gut = decw.tile([128, chi - clo], BF16, name="gut",
                                        tag="gut")
                        nc.sync.dma_start(
                            out=gut[:],
                            in_=guw_d[l, 128 * k:128 * (k + 1), clo:chi])
                        for j in range(jlo, jhi):
                            nc.tensor.matmul(
                                gu_ps[j][:], x2T[:, k, :],
                                gut[:, gu_off[j] - clo:gu_off[j] - clo + GU_CH[j]],
                                start=(k == 0), stop=(k == NKC - 1))
                    # consume this pass's psums: pairs (g, u)
                    for j in range(jlo, jhi, 2):
                        if j < 6:
                            w = GU_CH[j]
                            g_ap = gu_ps[j][:]
                            u_ap = gu_ps[j + 1][:]
                        else:  # combined [g3 | u3] chunk
                            w = 192
                            g_ap = gu_ps[6][:, 0:192]
                            u_ap = gu_ps[6][:, 192:384]
                        hcol = gu_off[j] // 2
                        sg = sc1.tile([DTOK, 512], BF16, name="sg", tag="sg")
                        nc.scalar.activation(sg[:, :w], g_ap, AF.Silu,
                                             scale=inv2[:])
                        uh = sc1.tile([DTOK, 512], BF16, name="uh", tag="uh")
                        nc.vector.tensor_scalar_mul(uh[:, :w], u_ap, inv2[:])
                        nc.vector.tensor_tensor(
                            out=h_sb[:, hcol:hcol + w],
                            in0=sg[:, :w], in1=uh[:, :w], op=OP.mult)
                # hT
                hT = decx.tile([128, DKC, DTOK], BF16, name="hT", tag="hT")
                for k in range(DKC):
                    klen = 128 if k < DKC - 1 else DK_LAST
                    hp = ktps.tile([128, DTOK], BF16, name="hp", tag="kt_ps")
                    nc.tensor.transpose(hp[:klen, :],
                                        h_sb[:, 128 * k:128 * k + klen],
                                        ident[:DTOK, :DTOK])
                    nc.vector.tensor_copy(hT[:klen, k, :], hp[:klen, :])
                # down
                dar = arb.tile([DTOK, H], F32, name="dar", tag="oar")
                for nns in [(0, 3), (3, 5)]:
                    d_ps = {nn: dps.tile([DTOK, 512], F32, name="d_ps", tag="acc")
                            for nn in range(*nns)}
                    for nn in range(*nns):
                        for k in range(DKC):
                            klen = 128 if k < DKC - 1 else DK_LAST
                            dwt = decw.tile([128, 512], BF16, name="dwt",
                                            tag="dwt")
                            nc.sync.dma_start(
                                out=dwt[:klen, :],
                                in_=dw_d[l, 128 * k:128 * k + klen,
                                         512 * nn:512 * (nn + 1)])
                            nc.tensor.matmul(d_ps[nn][:], hT[:klen, k, :],
                                             dwt[:klen, :],
                                             start=(k == 0), stop=(k == DKC - 1))
                    for nn in range(*nns):
                        nc.vector.tensor_copy(dar[:, 512 * nn:512 * (nn + 1)],
                                              d_ps[nn][:])
                din = ardr.tile([DTOK, H], F32, name="din", tag="oin")
                dout = ardr.tile([DTOK, H], F32, name="dout", tag="oout")
                nc.sync.dma_start(out=din[:], in_=dar[:])
                nc.gpsimd.collective_compute("AllReduce", OP.add,
                                             replica_groups=GROUPS,
                                             ins=[din[:]], outs=[dout[:]])
                das = arb.tile([DTOK, H], F32, name="das", tag="oas")
                nc.sync.dma_start(out=das[:], in_=dout[:])
                nc.vector.tensor_tensor(out=x_sb[:], in0=x_sb[:], in1=das[:],
                                        op=OP.add)

            # final norm (norm_w applied on host)
            xsqf = sc1.tile([DTOK, H], BF16, name="xsqf", tag="xsq")
            ssf = sc1.tile([DTOK, 1], F32, name="ssf", tag="ssx")
            nc.scalar.activation(xsqf[:], x_sb[:], AF.Square,
                                 accum_out=ssf[:])
            invf = _rsqrt(nc, sc1, ssf, H, DTOK, "fin")
            y_sb = arb.tile([DTOK, H], F32, name="y_sb", tag="oar")
            nc.vector.tensor_scalar_mul(y_sb[:], x_sb[:], invf[:])
            nc.sync.dma_start(out=y_d[:], in_=y_sb[:])


_NC_CACHE = None


def _get_nc():
    global _NC_CACHE
    if _NC_CACHE is None:
        _NC_CACHE = _build()
    return _NC_CACHE


def _prep_inputs(noise_embedding, target_hidden, position_ids, fc_w,
                 hidden_norm_w, q_w, k_w, v_w, o_w, qn_w, kn_w, gate_w, up_w,
                 down_w, ln1_w, ln2_w, norm_w):
    bf = ml_dtypes.bfloat16
    pos = np.asarray(position_ids)
    inv_freq = (1.0 / (THETA ** (np.arange(0, HD, 2, dtype=np.float64) / HD)))

    def cos_sin(p):  # p: (n,) positions -> cos/sin (n, HD) float32
        ang = p.astype(np.float64)[:, None] * inv_freq[None, :]
        c = np.cos(ang); s = np.sin(ang)
        return (np.concatenate([c, c], -1).astype(np.float32),
                np.concatenate([s, s], -1).astype(np.float32))

    qw = np.asarray(q_w); kw = np.asarray(k_w); vw = np.asarray(v_w)
    ow = np.asarray(o_w); gw = np.asarray(gate_w); uw = np.asarray(up_w)
    dw = np.asarray(down_w); fw = np.asarray(fc_w)
    ln1 = np.asarray(ln1_w); ln2 = np.asarray(ln2_w)
    hw = np.asarray(hidden_norm_w)
    qn = np.asarray(qn_w); kn = np.asarray(kn_w)
    th_in = np.asarray(target_hidden)
    ne = np.asarray(noise_embedding)

    in_maps = []
    # per-rank weight tensors (shared across the two DP groups)
    rank_data = []
    for t in range(TP):
        fcw_t = np.ascontiguousarray(
            fw[640 * t:640 * (t + 1), :].T).astype(bf)       # [12800, 640]
        kvw_t = np.empty((L, H, 256), np.float32)
        qkvw_t = np.empty((L, H, 896), np.float32)
        ow_t = np.empty((L, HSH, H), np.float32)
        guw_t = np.empty((L, H, GU2), np.float32)
        dw_t = np.empty((L, ISH, H), np.float32)
        for l in range(L):
            kslc = kw[l, HD * t:HD * (t + 1), :] * hw[None, :]
            vslc = vw[l, HD * t:HD * (t + 1), :] * hw[None, :]
            kvw_t[l] = np.concatenate([kslc, vslc], 0).T
            qs = qw[l, 640 * t:640 * (t + 1), :] * ln1[l][None, :]
            kds = kw[l, HD * t:HD * (t + 1), :] * ln1[l][None, :]
            vds = vw[l, HD * t:HD * (t + 1), :] * ln1[l][None, :]
            qkvw_t[l] = np.concatenate([qs, kds, vds], 0).T
            ow_t[l] = ow[l][:, 640 * t:640 * (t + 1)].T
            g = gw[l, ISH * t:ISH * (t + 1), :] * ln2[l][None, :]
            u = uw[l, ISH * t:ISH * (t + 1), :] * ln2[l][None, :]
            # interleave gate/up in 512-col pair chunks (g0,u0,g1,u1,g2,u2,g3,u3)
            parts = []
            for j, wdt in enumerate([512, 512, 512, 192]):
                o0 = 512 * j
                parts.append(g[o0:o0 + wdt, :])
                parts.append(u[o0:o0 + wdt, :])
            guw_t[l] = np.concatenate(parts, 0).T
            dw_t[l] = dw[l][:, ISH * t:ISH * (t + 1)].T
        rank_data.append(dict(
            fcw=fcw_t, kvw=kvw_t.astype(bf), qkvw=qkvw_t.astype(bf),
            ow=ow_t.astype(bf), guw=guw_t.astype(bf), dw=dw_t.astype(bf)))

    group_data = []
    for g in range(2):
        bsel = [2 * g, 2 * g + 1]
        xt_g = np.ascontiguousarray(
            th_in[bsel].transpose(2, 0, 1).reshape(FIN, TOK)).astype(bf)
        x0_g = np.ascontiguousarray(ne[bsel].reshape(DTOK, H), np.float32)
        ckc = np.empty((L, 2, CTX, HD), np.float32)
        skc = np.empty((L, 2, CTX, HD), np.float32)
        ckq = np.empty((L, DTOK, HD), np.float32)
        skq = np.empty((L, DTOK, HD), np.float32)
        ckd = np.empty((L, DTOK, HD), np.float32)
        skd = np.empty((L, DTOK, HD), np.float32)
        for bi, b in enumerate(bsel):
            cc, ss = cos_sin(pos[b, :CTX])
            cd, sd = cos_sin(pos[b, CTX:S])
            for l in range(L):
                knl = kn[l]; qnl = qn[l]
                # k tables: CK = kn*cos; SK[:64] = -kn[64:]*sin[:64],
                #           SK[64:] = kn[:64]*sin[64:]
                ckc[l, bi] = cc * knl[None, :]
                skc[l, bi, :, :64] = -ss[:, :64] * knl[None, 64:]
                skc[l, bi, :, 64:] = ss[:, 64:] * knl[None, :64]
                ckd[l, Q * bi:Q * (bi + 1)] = cd * knl[None, :]
                skd[l, Q * bi:Q * (bi + 1), :64] = -sd[:, :64] * knl[None, 64:]
                skd[l, Q * bi:Q * (bi + 1), 64:] = sd[:, 64:] * knl[None, :64]
                sc = 1.0 / np.sqrt(HD)
                ckq[l, Q * bi:Q * (bi + 1)] = cd * qnl[None, :] * sc
                skq[l, Q * bi:Q * (bi + 1), :64] = \
                    -sd[:, :64] * qnl[None, 64:] * sc
                skq[l, Q * bi:Q * (bi + 1), 64:] = \
                    sd[:, 64:] * qnl[None, :64] * sc
        group_data.append(dict(
            xt=xt_g, x0=x0_g, ckc=ckc.astype(bf), skc=skc.astype(bf),
            ckq=ckq.astype(bf), skq=skq.astype(bf), ckd=ckd.astype(bf),
            skd=skd.astype(bf)))

    for core in range(NCORES):
        g, t = core // TP, core % TP
        m = {}
        m.update(rank_data[t])
        m.update(group_data[g])
        in_maps.append(m)
    return in_maps


def kernel(**inputs):
    nc = _get_nc()
    in_maps = _prep_inputs(**inputs)
    res = bass_utils.run_bass_kernel_spmd(
        nc, in_maps, core_ids=list(range(NCORES)), trace=False)
    norm_w = np.asarray(inputs["norm_w"]).astype(np.float32)
    y0 = res.results[0]["y"]
    y1 = res.results[TP]["y"]
    y = np.concatenate([y0, y1], 0) * norm_w[None, :]
    return y.reshape(B, Q, H).astype(np.float32)



# revision 4
# speedup vs baseline: 1.4869x; 1.4869x over previous
"""DFlashDraftModel Trainium2 kernel — 8 NeuronCores, DP2 x TP4.

Sharding: cores 0-3 handle batches 0-1, cores 4-7 handle batches 2-3 (DP groups).
Within each group of 4 (TP): fc output-dim sharded (640/core) + AllGather;
q heads 5/core, kv heads 1/core, o input-sharded, gate/up 1728/core,
down input-sharded; AllReduce after o_proj and down_proj within the group.

fc and context-KV matmuls run in fp8e4m3 with DoubleRow perf mode (2 MACs/
cell/cycle); weights pre-scaled x16 on host, scale folded into the th
inv-rms factor. Everything else bf16 with fp32 PSUM accumulation. RMS-norm
scales are commuted through matmuls; per-token scalars that cancel in
downstream per-head RMS norms are skipped.

Context-KV build for layer l+1 is emitted between the decode-layer-l
collective stalls so the PE stays busy during AllReduce waits.
"""
import sys

if '/opt/trn_rl_repo' not in sys.path:
    sys.path.insert(0, '/opt/trn_rl_repo')

import numpy as np
import ml_dtypes

import concourse.bass as bass
import concourse.tile as tile
from concourse import bacc, mybir
from concourse import bass_utils
from concourse.masks import make_identity

BF16 = mybir.dt.bfloat16
F32 = mybir.dt.float32
F8 = mybir.dt.float8e4
AF = mybir.ActivationFunctionType
OP = mybir.AluOpType
DR = mybir.MatmulPerfMode.DoubleRow

L = 5; B = 4; Q = 16; CTX = 2048; S = CTX + Q
H = 2560; I = 6912; NH = 20; NKV = 4; HD = 128; NF = 5
EPS = 1e-6; THETA = 1000000.0

NCORES = 8
TP = 4                      # tensor-parallel group size
GROUPS = [[0, 1, 2, 3], [4, 5, 6, 7]]
TOK = 2 * CTX               # ctx tokens per core (2 batches)
NTC = TOK // 128            # 32 context token chunks per core
FIN = NF * H                # 12800
KF = FIN // 128             # 100 fc contraction chunks
HSH = H // TP               # 640  fc output shard per core
ISH = I // TP               # 1728 gate/up shard
QH = NH // TP               # 5 q heads per core
DTOK = 2 * Q                # 32 decoder tokens per core
NKC = 20                    # H/128 contraction chunks
NFC = 8                     # fc token chunks (512 each)
FCN = 512                   # fc token chunk width
GU2 = 2 * ISH               # 3456
# gate/up interleaved column chunks: [g0,u0,g1,u1,g2,u2,(g3|u3)]
GU_CH = [512, 512, 512, 512, 512, 512, 384]
DKC = 14                    # down contraction chunks: 13*128 + 64
DK_LAST = ISH - 13 * 128    # 64
FSC = 16.0                  # fp8 host prescale on fc_w and k/v weights
ISC2 = 1.0 / (FSC * FSC)    # combined descale for V path


def _build():
    nc = bacc.Bacc("TRN2", target_bir_lowering=False, debug=False,
                   enable_asserts=False, num_devices=NCORES)

    # ---- I/O ----
    xt_d = nc.dram_tensor("xt", [FIN, TOK], F8, kind="ExternalInput").ap()
    fcw_d = nc.dram_tensor("fcw", [FIN, HSH], F8, kind="ExternalInput").ap()
    kvw_d = nc.dram_tensor("kvw", [L, H, 256], F8, kind="ExternalInput").ap()
    qkvw_d = nc.dram_tensor("qkvw", [L, H, 896], BF16, kind="ExternalInput").ap()
    ow_d = nc.dram_tensor("ow", [L, HSH, H], BF16, kind="ExternalInput").ap()
    guw_d = nc.dram_tensor("guw", [L, H, GU2], BF16, kind="ExternalInput").ap()
    dw_d = nc.dram_tensor("dw", [L, ISH, H], BF16, kind="ExternalInput").ap()
    # packed ctx rope tables: [L, 2(b), 128(p), 16(c), 2(ck|sk), HD]
    tabc_d = nc.dram_tensor("tabc", [L, 2, 128, 16, 2, HD], BF16,
                            kind="ExternalInput").ap()
    ckq_d = nc.dram_tensor("ckq", [L, DTOK, HD], BF16, kind="ExternalInput").ap()
    skq_d = nc.dram_tensor("skq", [L, DTOK, HD], BF16, kind="ExternalInput").ap()
    ckd_d = nc.dram_tensor("ckd", [L, DTOK, HD], BF16, kind="ExternalInput").ap()
    skd_d = nc.dram_tensor("skd", [L, DTOK, HD], BF16, kind="ExternalInput").ap()
    x0_d = nc.dram_tensor("x0", [DTOK, H], F32, kind="ExternalInput").ap()
    y_d = nc.dram_tensor("y", [DTOK, H], F32, kind="ExternalOutput").ap()

    with tile.TileContext(nc) as tc:
        _emit(nc, tc, xt_d, fcw_d, kvw_d, qkvw_d, ow_d, guw_d, dw_d,
              tabc_d, ckq_d, skq_d, ckd_d, skd_d, x0_d, y_d)
    nc.compile()
    return nc


def _rsqrt(nc, pool, ss, n_mean, ptok, name):
    """rsqrt(ss/n_mean + EPS) for per-partition scalars ss [ptok,1] f32."""
    m = pool.tile([ptok, 1], F32, name=f"m_{name}")
    nc.vector.tensor_scalar(out=m[:], in0=ss[:], scalar1=1.0 / n_mean,
                            scalar2=EPS, op0=OP.mult, op1=OP.add)
    r = pool.tile([ptok, 1], F32, name=f"r_{name}")
    nc.vector.reciprocal(r[:], m[:])
    o = pool.tile([ptok, 1], F32, name=f"o_{name}")
    nc.scalar.sqrt(o[:], r[:])
    return o


def _rope(nc, pool, src, ck, sk, ptok, name):
    """out = src*ck + rotate_half_pair(src)*sk  (sign folded into sk).
    src/ck/sk: [ptok, 128] bf16 APs. Returns bf16 tile."""
    t1 = pool.tile([ptok, HD], BF16, name=f"t1_{name}")
    nc.vector.tensor_tensor(out=t1[:], in0=src, in1=ck, op=OP.mult)
    t2 = pool.tile([ptok, HD], BF16, name=f"t2_{name}")
    nc.vector.tensor_tensor(out=t2[:, 0:64], in0=src[:, 64:128],
                            in1=sk[:, 0:64], op=OP.mult)
    nc.vector.tensor_tensor(out=t2[:, 64:128], in0=src[:, 0:64],
                            in1=sk[:, 64:128], op=OP.mult)
    t3 = pool.tile([ptok, HD], BF16, name=f"t3_{name}")
    nc.vector.tensor_tensor(out=t3[:], in0=t1[:], in1=t2[:], op=OP.add)
    return t3


def _emit(nc, tc, xt_d, fcw_d, kvw_d, qkvw_d, ow_d, guw_d, dw_d,
          tabc_d, ckq_d, skq_d, ckd_d, skd_d, x0_d, y_d):
    with tc.tile_pool(name="glob", bufs=1) as glob, \
         tc.tile_pool(name="dramg", bufs=1, space="DRAM") as dram:
        # ---------------- long-lived tiles ----------------
        ident = glob.tile([128, 128], BF16, name="ident")
        make_identity(nc, ident[:])
        # shifted identity: ident_sh[16+j, j] = 1 (selects rows 16:32)
        ident_sh = glob.tile([128, 128], BF16, name="ident_sh")
        nc.gpsimd.memset(ident_sh[:], 0.0)
        nc.gpsimd.affine_select(out=ident_sh[:], in_=ident_sh[:],
                                compare_op=OP.not_equal, fill=1.0, base=-16,
                                pattern=[[-1, 128]], channel_multiplier=1)
        ones = glob.tile([128, 1], BF16, name="ones")
        nc.vector.memset(ones[:], 1.0)
        i1f = glob.tile([1, 1], F32, name="i1f")
        nc.vector.memset(i1f[:], 1.0)
        x_sb = glob.tile([DTOK, H], F32, name="x_sb")
        nc.sync.dma_start(out=x_sb[:], in_=x0_d[:])
        invK = glob.tile([128, NTC], F32, name="invK")   # rsqrt (K path)
        invV = glob.tile([128, NTC], F32, name="invV")   # rsqrt/256 (V path)

        th_n = [dram.tile([H, FCN], F8, name=f"th{n}") for n in range(NFC)]
        ssq_in = dram.tile([1, TOK], F32, name="ssq_in")
        ssq_out = dram.tile([1, TOK], F32, name="ssq_out")

        # ---------------- Phase 1: fc (fp8 DoubleRow) ----------------
        with tc.tile_pool(name="fcw_pool", bufs=1) as fcwp, \
             tc.tile_pool(name="xtp", bufs=6) as xtp, \
             tc.tile_pool(name="fcps", bufs=6, space="PSUM") as fcps, \
             tc.tile_pool(name="ssqps", bufs=2, space="PSUM") as ssqps, \
             tc.tile_pool(name="fcev", bufs=4) as fcev, \
             tc.tile_pool(name="ssqrp", bufs=1) as ssqrp, \
             tc.tile_pool(name="agp", bufs=2, space="DRAM") as agp:
            fcw_sb = fcwp.tile([128, KF, HSH], F8)
            ssq_row = ssqrp.tile([1, TOK], F32, name="ssq_row")
            fcw_r = fcw_d.rearrange("(kk p) m -> p kk m", p=128)
            for qq in range(4):
                nc.sync.dma_start(
                    out=fcw_sb[:, 25 * qq:25 * (qq + 1), :],
                    in_=fcw_r[:, 25 * qq:25 * (qq + 1), :])
            xt_r = xt_d.rearrange("(kk p) m -> p kk m", p=128)
            for n in range(NFC):
                psF = [fcps.tile([128, FCN], F32, name="psF", tag="psF")
                       for _ in range(5)]
                for k in range(KF // 2):
                    xt_t = xtp.tile([128, 2, FCN], F8, name="xt_t", tag="xt_t")
                    nc.sync.dma_start(
                        out=xt_t[:],
                        in_=xt_r[:, 2 * k:2 * k + 2, FCN * n:FCN * (n + 1)])
                    for m in range(5):
                        nc.tensor.matmul(
                            psF[m][:],
                            fcw_sb[:, 2 * k:2 * k + 2, 128 * m:128 * (m + 1)],
                            xt_t[:], start=(k == 0), stop=(k == KF // 2 - 1),
                            perf_mode=DR)
                ag_in = agp.tile([HSH, FCN], F8, name="ag_in", tag="ag_in")
                ssq_ps = ssqps.tile([1, FCN], F32, name="ssq_ps", tag="ssq_ps")
                for m in range(5):
                    th_f8 = fcev.tile([128, FCN], F8, name="th_f8", tag="th_f8")
                    nc.vector.tensor_copy(th_f8[:], psF[m][:])
                    nc.sync.dma_start(out=ag_in[128 * m:128 * (m + 1), :],
                                      in_=th_f8[:])
                    th2 = fcev.tile([128, FCN], BF16, name="th2", tag="th2")
                    nc.scalar.square(th2[:], psF[m][:])
                    nc.tensor.matmul(ssq_ps[:], ones[:], th2[:],
                                     start=(m == 0), stop=(m == 4))
                nc.vector.tensor_copy(ssq_row[:, FCN * n:FCN * (n + 1)], ssq_ps[:])
                nc.gpsimd.collective_compute(
                    "AllGather", OP.bypass, replica_groups=GROUPS,
                    ins=[ag_in[:]], outs=[th_n[n][:]])
            # sum-of-squares all-reduce + inv_rms in [128, NTC] layout
            nc.sync.dma_start(out=ssq_in[:], in_=ssq_row[:])
            nc.gpsimd.collective_compute(
                "AllReduce", OP.add, replica_groups=GROUPS,
                ins=[ssq_in[:]], outs=[ssq_out[:]])

        # inv_rms of th in [128, NTC] per-partition layout.
        # ssq is from the x16-scaled fp8 matmul: mean_true = ssq/(FSC^2*H).
        # invK = rsqrt(mean+eps) (scale cancels in downstream per-head rms);
        # invV = invK/FSC^2 (exact descale for the V path).
        with tc.tile_pool(name="ivps", bufs=2, space="PSUM") as ivps, \
             tc.tile_pool(name="ivtmp", bufs=2) as ivtmp:
            sro = ivtmp.tile([1, TOK], F32, name="sro")
            nc.sync.dma_start(out=sro[:], in_=ssq_out[:])
            ssT = ivtmp.tile([128, NTC], F32, name="ssT")
            for j in range(NTC):
                tp = ivps.tile([128, 1], F32, name="tp_iv", tag="tp_iv")
                nc.tensor.matmul(tp[:], sro[:, 128 * j:128 * (j + 1)],
                                 i1f[:], start=True, stop=True)
                nc.vector.tensor_copy(ssT[:, j:j + 1], tp[:])
            m = ivtmp.tile([128, NTC], F32, name="m_iv")
            nc.vector.tensor_scalar(out=m[:], in0=ssT[:],
                                    scalar1=1.0 / (FSC * FSC * H),
                                    scalar2=EPS, op0=OP.mult, op1=OP.add)
            r = ivtmp.tile([128, NTC], F32, name="r_iv")
            nc.vector.reciprocal(r[:], m[:])
            nc.scalar.sqrt(invK[:], r[:])
            nc.vector.tensor_scalar(out=invV[:], in0=invK[:], scalar1=ISC2,
                                    scalar2=0.0, op0=OP.mult, op1=OP.add)

        # ---------------- Phases 2+3: per-layer ----------------
        with tc.tile_pool(name="kvwp", bufs=2) as kvwp, \
             tc.tile_pool(name="ktc", bufs=2) as ktcp, \
             tc.tile_pool(name="vcp", bufs=2) as vcp, \
             tc.tile_pool(name="tabp", bufs=2) as tabp, \
             tc.tile_pool(name="tht", bufs=4) as thtp, \
             tc.tile_pool(name="kvps", bufs=2, space="PSUM") as kvps, \
             tc.tile_pool(name="ktps", bufs=2, space="PSUM") as ktps, \
             tc.tile_pool(name="kvpost", bufs=3) as kvpost, \
             tc.tile_pool(name="sc1", bufs=2) as sc1, \
             tc.tile_pool(name="arb", bufs=1) as arb, \
             tc.tile_pool(name="decw", bufs=3) as decw, \
             tc.tile_pool(name="dps", bufs=4, space="PSUM") as dps, \
             tc.tile_pool(name="attp", bufs=3) as attp, \
             tc.tile_pool(name="decx", bufs=2) as decx, \
             tc.tile_pool(name="ardr", bufs=2, space="DRAM") as ardr:

            kvstate = {}

            def kv_build(l, t_lo, t_hi):
                """Context K/V for layer l, token chunks [t_lo, t_hi)."""
                if t_lo == 0:
                    kvw_sb = kvwp.tile([128, NKC, 256], F8, name="kvw_sb",
                                       tag="kvw")
                    nc.sync.dma_start(
                        out=kvw_sb[:],
                        in_=kvw_d[l].rearrange("(kk p) c -> p kk c", p=128))
                    KT_c = ktcp.tile([128, 2, S], BF16, name="KT_c", tag="KT")
                    V_c = vcp.tile([128, 2, 17, HD], BF16, name="V_c", tag="V")
                    kvstate[l] = (kvw_sb, KT_c, V_c)
                kvw_sb, KT_c, V_c = kvstate[l]
                # half-batch rope table: [128, 16, 2, HD]
                b0 = t_lo // 16
                tab = tabp.tile([128, 16, 2, HD], BF16, name="tab", tag="tab")
                nc.sync.dma_start(out=tab[:], in_=tabc_d[l, b0])
                for t_ in range(t_lo, t_hi):
                    b, c = t_ // 16, t_ % 16
                    thT_t = thtp.tile([128, NKC, 128], F8, name="thT_t",
                                      tag="thT")
                    nc.sync.dma_start(
                        out=thT_t[:],
                        in_=th_n[t_ // 4].rearrange(
                            "(kk p) (cc w) -> p kk cc w", p=128, w=128)
                            [:, :, t_ % 4, :])
                    kv_ps = kvps.tile([128, 256], F32, name="kv_ps", tag="kv_ps")
                    for k in range(NKC // 2):
                        nc.tensor.matmul(kv_ps[:], thT_t[:, 2 * k:2 * k + 2, :],
                                         kvw_sb[:, 2 * k:2 * k + 2, :],
                                         start=(k == 0), stop=(k == NKC // 2 - 1),
                                         perf_mode=DR)
                    ivk = invK[:, t_:t_ + 1]
                    nc.vector.tensor_scalar_mul(V_c[:, b, c, :],
                                                kv_ps[:, 128:256],
                                                invV[:, t_:t_ + 1])
                    kbf = kvpost.tile([128, HD], BF16, name="kbf", tag="kbf")
                    nc.vector.tensor_scalar_mul(kbf[:], kv_ps[:, 0:128], ivk)
                    k2t = kvpost.tile([128, HD], BF16, name="k2t", tag="k2t")
                    ks = kvpost.tile([128, 1], F32, name="ks", tag="ks")
                    nc.scalar.activation(k2t[:], kbf[:], AF.Square,
                                         accum_out=ks[:])
                    rs = _rsqrt(nc, kvpost, ks, HD, 128, "kh")
                    khn = kvpost.tile([128, HD], BF16, name="khn", tag="khn")
                    nc.vector.tensor_scalar_mul(khn[:], kbf[:], rs[:])
                    kr = _rope(nc, kvpost, khn[:], tab[:, c, 0, :],
                               tab[:, c, 1, :], 128, "kc")
                    kt_ps = ktps.tile([128, 128], BF16, name="kt_ps", tag="kt_ps")
                    nc.tensor.transpose(kt_ps[:], kr[:], ident[:])
                    nc.vector.tensor_copy(KT_c[:, b, 128 * c:128 * (c + 1)],
                                          kt_ps[:])

            kv_build(0, 0, 16)
            kv_build(0, 16, 32)

            for l in range(L):
                _, KT_c, V_c = kvstate.pop(l)
                # ===== decode part 1: attention =====
                # inv_rms1 of x (needed for vd only; per-token scalars cancel
                # in q/kd per-head rms)
                xsq = sc1.tile([DTOK, H], BF16, name="xsq", tag="xsq")
                ssx = sc1.tile([DTOK, 1], F32, name="ssx", tag="ssx")
                nc.scalar.activation(xsq[:], x_sb[:], AF.Square, accum_out=ssx[:])
                inv1 = _rsqrt(nc, sc1, ssx, H, DTOK, "in1")
                # xT (raw x, bf16)
                xb = sc1.tile([DTOK, H], BF16, name="xb", tag="xb")
                nc.vector.tensor_copy(xb[:], x_sb[:])
                xT = decx.tile([128, NKC, DTOK], BF16, name="xT", tag="xT")
                for k in range(NKC):
                    tp = ktps.tile([128, DTOK], BF16, name="tp_x", tag="kt_ps")
                    nc.tensor.transpose(tp[:], xb[:, 128 * k:128 * (k + 1)],
                                        ident[:DTOK, :DTOK])
                    nc.vector.tensor_copy(xT[:, k, :], tp[:])
                # qkv projection
                q_ps = dps.tile([DTOK, 512], F32, name="q_ps", tag="acc")
                q_ps2 = dps.tile([DTOK, 384], F32, name="q_ps2", tag="acc")
                for k in range(NKC):
                    wt = decw.tile([128, 896], BF16, name="qkvwt", tag="qkvwt")
                    nc.sync.dma_start(out=wt[:],
                                      in_=qkvw_d[l, 128 * k:128 * (k + 1), :])
                    nc.tensor.matmul(q_ps[:], xT[:, k, :], wt[:, 0:512],
                                     start=(k == 0), stop=(k == NKC - 1))
                    nc.tensor.matmul(q_ps2[:], xT[:, k, :], wt[:, 512:896],
                                     start=(k == 0), stop=(k == NKC - 1))
                # q: per-head rms (raw — per-token scale cancels), rope
                q1 = sc1.tile([DTOK, QH * HD], BF16, name="q1", tag="q1")
                nc.vector.tensor_copy(q1[:, 0:512], q_ps[:])
                nc.vector.tensor_copy(q1[:, 512:640], q_ps2[:, 0:128])
                ckq_sb = sc1.tile([DTOK, HD], BF16, name="ckq_sb", tag="ckq")
                skq_sb = sc1.tile([DTOK, HD], BF16, name="skq_sb", tag="skq")
                nc.sync.dma_start(out=ckq_sb[:], in_=ckq_d[l])
                nc.sync.dma_start(out=skq_sb[:], in_=skq_d[l])
                qr = sc1.tile([DTOK, QH * HD], BF16, name="qr", tag="qr")
                for h in range(QH):
                    hs = slice(128 * h, 128 * (h + 1))
                    q2h = sc1.tile([DTOK, HD], BF16, name="q2h", tag="q2h")
                    qsh = sc1.tile([DTOK, 1], F32, name="qsh", tag="qsh")
                    nc.scalar.activation(q2h[:], q1[:, hs], AF.Square,
                                         accum_out=qsh[:])
                    rqh = _rsqrt(nc, sc1, qsh, HD, DTOK, f"qh{h}")
                    qhn = sc1.tile([DTOK, HD], BF16, name="qhn", tag="qhn")
                    nc.vector.tensor_scalar_mul(qhn[:], q1[:, hs], rqh[:])
                    qrh = _rope(nc, sc1, qhn[:], ckq_sb[:], skq_sb[:], DTOK, "q")
                    nc.vector.tensor_copy(qr[:, hs], qrh[:])
                # qT per batch
                qT = [sc1.tile([128, QH * Q], BF16, name=f"qT{b}", tag=f"qT{b}")
                      for b in range(2)]
                for b in range(2):
                    sel = ident[:DTOK, 0:Q] if b == 0 else ident_sh[:DTOK, 0:Q]
                    for h in range(QH):
                        tqp = ktps.tile([128, Q], F32, name="tqp", tag="kt_ps")
                        nc.tensor.matmul(tqp[:], qr[:, 128 * h:128 * (h + 1)],
                                         sel, start=True, stop=True)
                        nc.vector.tensor_copy(qT[b][:, Q * h:Q * (h + 1)], tqp[:])
                # kd (raw — scale cancels in head rms) / vd (needs inv1)
                k1d = sc1.tile([DTOK, HD], BF16, name="k1d", tag="k1d")
                nc.vector.tensor_copy(k1d[:], q_ps2[:, 128:256])
                k2d = sc1.tile([DTOK, HD], BF16, name="k2d", tag="k2d")
                ksd = sc1.tile([DTOK, 1], F32, name="ksd", tag="ksd")
                nc.scalar.activation(k2d[:], k1d[:], AF.Square,
                                     accum_out=ksd[:])
                rsd = _rsqrt(nc, sc1, ksd, HD, DTOK, "kd")
                khd = sc1.tile([DTOK, HD], BF16, name="khd", tag="khd")
                nc.vector.tensor_scalar_mul(khd[:], k1d[:], rsd[:])
                ckd_sb = sc1.tile([DTOK, HD], BF16, name="ckd_sb", tag="ckd")
                skd_sb = sc1.tile([DTOK, HD], BF16, name="skd_sb", tag="skd")
                nc.sync.dma_start(out=ckd_sb[:], in_=ckd_d[l])
                nc.sync.dma_start(out=skd_sb[:], in_=skd_d[l])
                krd = _rope(nc, sc1, khd[:], ckd_sb[:], skd_sb[:], DTOK, "kd")
                vd = sc1.tile([DTOK, HD], BF16, name="vd", tag="vd")
                nc.vector.tensor_scalar_mul(vd[:], q_ps2[:, 256:384], inv1[:])
                for b in range(2):
                    sel = ident[:DTOK, 0:Q] if b == 0 else ident_sh[:DTOK, 0:Q]
                    kdp = ktps.tile([128, Q], F32, name="kdp", tag="kt_ps")
                    nc.tensor.matmul(kdp[:], krd[:], sel, start=True, stop=True)
                    nc.vector.tensor_copy(KT_c[:, b, CTX:S], kdp[:])
                    # vd: partition shift via sbuf->sbuf dma
                    nc.sync.dma_start(out=V_c[0:Q, b, 16, :],
                                      in_=vd[Q * b:Q * (b + 1), :])
                # attention per batch
                attn_cat = sc1.tile([128, 2 * QH * Q], BF16, name="attn_cat",
                                    tag="attn_cat")
                for b in range(2):
                    rs_ps = dps.tile([1, QH * Q], F32, name="rs_ps", tag="acc")
                    ao_ps = dps.tile([QH * Q, HD], F32, name="ao_ps", tag="acc")
                    for s_ in range(17):
                        klen = 128 if s_ < 16 else Q
                        sc_ps = dps.tile([128, QH * Q], F32, name="sc_ps", tag="acc")
                        nc.tensor.matmul(
                            sc_ps[:klen, :],
                            KT_c[:, b, 128 * s_:128 * s_ + klen], qT[b][:],
                            start=True, stop=True)
                        at = attp.tile([128, QH * Q], BF16, name="at", tag="at")
                        nc.scalar.activation(at[:klen, :], sc_ps[:klen, :], AF.Exp)
                        nc.tensor.matmul(rs_ps[:], ones[:klen, :], at[:klen, :],
                                         start=(s_ == 0), stop=(s_ == 16))
                        nc.tensor.matmul(ao_ps[:], at[:klen, :],
                                         V_c[:klen, b, s_, :],
                                         start=(s_ == 0), stop=(s_ == 16))
                    rs_sb = sc1.tile([1, QH * Q], F32, name="rs_sb", tag="rs_sb")
                    nc.vector.tensor_copy(rs_sb[:], rs_ps[:])
                    rsT_ps = ktps.tile([QH * Q, 1], F32, name="rsT_ps",
                                       tag="kt_ps")
                    nc.tensor.matmul(rsT_ps[:], rs_sb[:], i1f[:],
                                     start=True, stop=True)
                    rinv = sc1.tile([QH * Q, 1], F32, name="rinv", tag="rinv")
                    nc.vector.reciprocal(rinv[:], rsT_ps[:])
                    aob = sc1.tile([QH * Q, HD], BF16, name="aob", tag="aob")
                    nc.vector.tensor_scalar_mul(aob[:], ao_ps[:], rinv[:])
                    aoT_ps = ktps.tile([128, QH * Q], BF16, name="aoT_ps",
                                       tag="kt_ps")
                    nc.tensor.transpose(aoT_ps[:], aob[:], ident[:QH * Q, :QH * Q])
                    nc.vector.tensor_copy(
                        attn_cat.rearrange("p (h bt) -> p h bt", bt=2 * Q)
                            [:, :, Q * b:Q * (b + 1)],
                        aoT_ps.rearrange("p (h t) -> p h t", t=Q)[:])
                # o projection (h-outer within nn-group, batched weight DMA)
                oar = arb.tile([DTOK, H], F32, name="oar", tag="oar")
                for nns in [(0, 3), (3, 5)]:
                    clo, chi = 512 * nns[0], 512 * nns[1]
                    o_ps = {nn: dps.tile([DTOK, 512], F32, name="o_ps", tag="acc")
                            for nn in range(*nns)}
                    for h in range(QH):
                        owt = decw.tile([128, 1536], BF16, name="owt", tag="owt")
                        nc.sync.dma_start(
                            out=owt[:, :chi - clo],
                            in_=ow_d[l, 128 * h:128 * (h + 1), clo:chi])
                        for nn in range(*nns):
                            nc.tensor.matmul(
                                o_ps[nn][:], attn_cat[:, 32 * h:32 * (h + 1)],
                                owt[:, 512 * nn - clo:512 * (nn + 1) - clo],
                                start=(h == 0), stop=(h == QH - 1))
                    for nn in range(*nns):
                        nc.vector.tensor_copy(oar[:, 512 * nn:512 * (nn + 1)],
                                              o_ps[nn][:])
                oin = ardr.tile([DTOK, H], F32, name="oin", tag="oin")
                oout = ardr.tile([DTOK, H], F32, name="oout", tag="oout")
                nc.sync.dma_start(out=oin[:], in_=oar[:])
                nc.gpsimd.collective_compute("AllReduce", OP.add,
                                             replica_groups=GROUPS,
                                             ins=[oin[:]], outs=[oout[:]])
                # ---- fill the o-AllReduce stall with next-layer KV build
                if l + 1 < L:
                    kv_build(l + 1, 0, 16)
                # ===== decode part 2: MLP =====
                oas = arb.tile([DTOK, H], F32, name="oas", tag="oas")
                nc.sync.dma_start(out=oas[:], in_=oout[:])
                nc.vector.tensor_tensor(out=x_sb[:], in0=x_sb[:], in1=oas[:],
                                        op=OP.add)
                # inv_rms2 + x2T
                xsq2 = sc1.tile([DTOK, H], BF16, name="xsq2", tag="xsq")
                ssx2 = sc1.tile([DTOK, 1], F32, name="ssx2", tag="ssx")
                nc.scalar.activation(xsq2[:], x_sb[:], AF.Square,
                                     accum_out=ssx2[:])
                inv2 = _rsqrt(nc, sc1, ssx2, H, DTOK, "in2")
                xb2 = sc1.tile([DTOK, H], BF16, name="xb2", tag="xb")
                nc.vector.tensor_copy(xb2[:], x_sb[:])
                x2T = decx.tile([128, NKC, DTOK], BF16, name="x2T", tag="x2T")
                for k in range(NKC):
                    tp2 = ktps.tile([128, DTOK], BF16, name="tp_x2", tag="kt_ps")
                    nc.tensor.transpose(tp2[:], xb2[:, 128 * k:128 * (k + 1)],
                                        ident[:DTOK, :DTOK])
                    nc.vector.tensor_copy(x2T[:, k, :], tp2[:])
                # gate/up
                h_sb = sc1.tile([DTOK, ISH], BF16, name="h_sb", tag="h_sb")
                gu_off = [0, 512, 1024, 1536, 2048, 2560, 3072]
                for jlo, jhi in [(0, 4), (4, 7)]:
                    clo, chi = gu_off[jlo], gu_off[jhi - 1] + GU_CH[jhi - 1]
                    gu_ps = {j: dps.tile([DTOK, GU_CH[j]], F32, name=f"gu{j}",
                                         tag="acc") for j in range(jlo, jhi)}
                    for k in range(NKC):
                        gut = decw.tile([128, chi - clo], BF16, name="gut",
                                        tag="gut")
                        nc.sync.dma_start(
                            out=gut[:],
                            in_=guw_d[l, 128 * k:128 * (k + 1), clo:chi])
                        for j in range(jlo, jhi):
                            nc.tensor.matmul(
                                gu_ps[j][:], x2T[:, k, :],
                                gut[:, gu_off[j] - clo:gu_off[j] - clo + GU_CH[j]],
                                start=(k == 0), stop=(k == NKC - 1))
                    # consume this pass's psums: pairs (g, u)
                    for j in range(jlo, jhi, 2):
                        if j < 6:
                            w = GU_CH[j]
                            g_ap = gu_ps[j][:]
                            u_ap = gu_ps[j + 1][:]
                        else:  # combined [g3 | u3] chunk
                            w = 192
                            g_ap = gu_ps[6][:, 0:192]
                            u_ap = gu_ps[6][:, 192:384]
                        hcol = gu_off[j] // 2
                        sg = sc1.tile([DTOK, 512], BF16, name="sg", tag="sg")
                        nc.scalar.activation(sg[:, :w], g_ap, AF.Silu,
                                             scale=inv2[:])
                        uh = sc1.tile([DTOK, 512], BF16, name="uh", tag="uh")
                        nc.vector.tensor_scalar_mul(uh[:, :w], u_ap, inv2[:])
                        nc.vector.tensor_tensor(
                            out=h_sb[:, hcol:hcol + w],
                            in0=sg[:, :w], in1=uh[:, :w], op=OP.mult)
                # hT
                hT = decx.tile([128, DKC, DTOK], BF16, name="hT", tag="hT")
                for k in range(DKC):
                    klen = 128 if k < DKC - 1 else DK_LAST
                    hp = ktps.tile([128, DTOK], BF16, name="hp", tag="kt_ps")
                    nc.tensor.transpose(hp[:klen, :],
                                        h_sb[:, 128 * k:128 * k + klen],
                                        ident[:DTOK, :DTOK])
                    nc.vector.tensor_copy(hT[:klen, k, :], hp[:klen, :])
                # down (k-outer within nn-group, batched weight DMA)
                dar = arb.tile([DTOK, H], F32, name="dar", tag="oar")
                for nns in [(0, 3), (3, 5)]:
                    clo, chi = 512 * nns[0], 512 * nns[1]
                    d_ps = {nn: dps.tile([DTOK, 512], F32, name="d_ps", tag="acc")
                            for nn in range(*nns)}
                    for k in range(DKC):
                        klen = 128 if k < DKC - 1 else DK_LAST
                        dwt = decw.tile([128, 1536], BF16, name="dwt", tag="dwt")
                        nc.sync.dma_start(
                            out=dwt[:klen, :chi - clo],
                            in_=dw_d[l, 128 * k:128 * k + klen, clo:chi])
                        for nn in range(*nns):
                            nc.tensor.matmul(
                                d_ps[nn][:], hT[:klen, k, :],
                                dwt[:klen, 512 * nn - clo:512 * (nn + 1) - clo],
                                start=(k == 0), stop=(k == DKC - 1))
                    for nn in range(*nns):
                        nc.vector.tensor_copy(dar[:, 512 * nn:512 * (nn + 1)],
                                              d_ps[nn][:])
                din = ardr.tile([DTOK, H], F32, name="din", tag="oin")
                dout = ardr.tile([DTOK, H], F32, name="dout", tag="oout")
                nc.sync.dma_start(out=din[:], in_=dar[:])
                nc.gpsimd.collective_compute("AllReduce", OP.add,
                                             replica_groups=GROUPS,
                                             ins=[din[:]], outs=[dout[:]])
                # ---- fill the down-AllReduce stall with next-layer KV build
                if l + 1 < L:
                    kv_build(l + 1, 16, 32)
                das = arb.tile([DTOK, H], F32, name="das", tag="oas")
                nc.sync.dma_start(out=das[:], in_=dout[:])
                nc.vector.tensor_tensor(out=x_sb[:], in0=x_sb[:], in1=das[:],
                                        op=OP.add)

            # final norm (norm_w applied on host)
            xsqf = sc1.tile([DTOK, H], BF16, name="xsqf", tag="xsq")
            ssf = sc1.tile([DTOK, 1], F32, name="ssf", tag="ssx")
            nc.scalar.activation(xsqf[:], x_sb[:], AF.Square,
                                 accum_out=ssf[:])
            invf = _rsqrt(nc, sc1, ssf, H, DTOK, "fin")
            y_sb = arb.tile([DTOK, H], F32, name="y_sb", tag="oar")
            nc.vector.tensor_scalar_mul(y_sb[:], x_sb[:], invf[:])
            nc.sync.dma_start(out=y_d[:], in_=y_sb[:])


_NC_CACHE = None


def _get_nc():
    global _NC_CACHE
    if _NC_CACHE is None:
        _NC_CACHE = _build()
    return _NC_CACHE


def _prep_inputs(noise_embedding, target_hidden, position_ids, fc_w,
                 hidden_norm_w, q_w, k_w, v_w, o_w, qn_w, kn_w, gate_w, up_w,
                 down_w, ln1_w, ln2_w, norm_w):
    bf = ml_dtypes.bfloat16
    f8 = ml_dtypes.float8_e4m3
    pos = np.asarray(position_ids)
    inv_freq = (1.0 / (THETA ** (np.arange(0, HD, 2, dtype=np.float64) / HD)))

    def cos_sin(p):  # p: (n,) positions -> cos/sin (n, HD) float32
        ang = p.astype(np.float64)[:, None] * inv_freq[None, :]
        c = np.cos(ang); s = np.sin(ang)
        return (np.concatenate([c, c], -1).astype(np.float32),
                np.concatenate([s, s], -1).astype(np.float32))

    qw = np.asarray(q_w); kw = np.asarray(k_w); vw = np.asarray(v_w)
    ow = np.asarray(o_w); gw = np.asarray(gate_w); uw = np.asarray(up_w)
    dw = np.asarray(down_w); fw = np.asarray(fc_w)
    ln1 = np.asarray(ln1_w); ln2 = np.asarray(ln2_w)
    hw = np.asarray(hidden_norm_w)
    qn = np.asarray(qn_w); kn = np.asarray(kn_w)
    th_in = np.asarray(target_hidden)
    ne = np.asarray(noise_embedding)

    in_maps = []
    # per-rank weight tensors (shared across the two DP groups)
    rank_data = []
    for t in range(TP):
        fcw_t = np.ascontiguousarray(
            fw[640 * t:640 * (t + 1), :].T * FSC).astype(f8)   # [12800, 640]
        kvw_t = np.empty((L, H, 256), np.float32)
        qkvw_t = np.empty((L, H, 896), np.float32)
        ow_t = np.empty((L, HSH, H), np.float32)
        guw_t = np.empty((L, H, GU2), np.float32)
        dw_t = np.empty((L, ISH, H), np.float32)
        for l in range(L):
            kslc = kw[l, HD * t:HD * (t + 1), :] * hw[None, :] * FSC
            vslc = vw[l, HD * t:HD * (t + 1), :] * hw[None, :] * FSC
            kvw_t[l] = np.concatenate([kslc, vslc], 0).T
            qs = qw[l, 640 * t:640 * (t + 1), :] * ln1[l][None, :]
            kds = kw[l, HD * t:HD * (t + 1), :] * ln1[l][None, :]
            vds = vw[l, HD * t:HD * (t + 1), :] * ln1[l][None, :]
            qkvw_t[l] = np.concatenate([qs, kds, vds], 0).T
            ow_t[l] = ow[l][:, 640 * t:640 * (t + 1)].T
            g = gw[l, ISH * t:ISH * (t + 1), :] * ln2[l][None, :]
            u = uw[l, ISH * t:ISH * (t + 1), :] * ln2[l][None, :]
            # interleave gate/up in 512-col pair chunks (g0,u0,g1,u1,g2,u2,g3,u3)
            parts = []
            for j, wdt in enumerate([512, 512, 512, 192]):
                o0 = 512 * j
                parts.append(g[o0:o0 + wdt, :])
                parts.append(u[o0:o0 + wdt, :])
            guw_t[l] = np.concatenate(parts, 0).T
            dw_t[l] = dw[l][:, ISH * t:ISH * (t + 1)].T
        rank_data.append(dict(
            fcw=fcw_t, kvw=kvw_t.astype(f8), qkvw=qkvw_t.astype(bf),
            ow=ow_t.astype(bf), guw=guw_t.astype(bf), dw=dw_t.astype(bf)))

    group_data = []
    for g in range(2):
        bsel = [2 * g, 2 * g + 1]
        xt_g = np.ascontiguousarray(
            th_in[bsel].transpose(2, 0, 1).reshape(FIN, TOK)).astype(f8)
        x0_g = np.ascontiguousarray(ne[bsel].reshape(DTOK, H), np.float32)
        # packed ctx tables [L, 2, 128, 16, 2, HD]
        tabc = np.empty((L, 2, 128, 16, 2, HD), np.float32)
        ckq = np.empty((L, DTOK, HD), np.float32)
        skq = np.empty((L, DTOK, HD), np.float32)
        ckd = np.empty((L, DTOK, HD), np.float32)
        skd = np.empty((L, DTOK, HD), np.float32)
        for bi, b in enumerate(bsel):
            cc, ss = cos_sin(pos[b, :CTX])
            cd, sd = cos_sin(pos[b, CTX:S])
            for l in range(L):
                knl = kn[l]; qnl = qn[l]
                # k tables: CK = kn*cos; SK[:64] = -kn[64:]*sin[:64],
                #           SK[64:] = kn[:64]*sin[64:]
                ckc_l = cc * knl[None, :]
                skc_l = np.empty_like(ss)
                skc_l[:, :64] = -ss[:, :64] * knl[None, 64:]
                skc_l[:, 64:] = ss[:, 64:] * knl[None, :64]
                # pack: tabc[l, bi, p, c, 0|1, :] = ck|sk row c*128+p
                tabc[l, bi, :, :, 0, :] = \
                    ckc_l.reshape(16, 128, HD).transpose(1, 0, 2)
                tabc[l, bi, :, :, 1, :] = \
                    skc_l.reshape(16, 128, HD).transpose(1, 0, 2)
                ckd[l, Q * bi:Q * (bi + 1)] = cd * knl[None, :]
                skd[l, Q * bi:Q * (bi + 1), :64] = -sd[:, :64] * knl[None, 64:]
                skd[l, Q * bi:Q * (bi + 1), 64:] = sd[:, 64:] * knl[None, :64]
                sc = 1.0 / np.sqrt(HD)
                ckq[l, Q * bi:Q * (bi + 1)] = cd * qnl[None, :] * sc
                skq[l, Q * bi:Q * (bi + 1), :64] = \
                    -sd[:, :64] * qnl[None, 64:] * sc
                skq[l, Q * bi:Q * (bi + 1), 64:] = \
                    sd[:, 64:] * qnl[None, :64] * sc
        group_data.append(dict(
            xt=xt_g, x0=x0_g, tabc=tabc.astype(bf),
            ckq=ckq.astype(bf), skq=skq.astype(bf), ckd=ckd.astype(bf),
            skd=skd.astype(bf)))

    for core in range(NCORES):
        g, t = core // TP, core % TP
        m = {}
        m.update(rank_data[t])
        m.update(group_data[g])
        in_maps.append(m)
    return in_maps


def kernel(**inputs):
    nc = _get_nc()
    in_maps = _prep_inputs(**inputs)
    res = bass_utils.run_bass_kernel_spmd(
        nc, in_maps, core_ids=list(range(NCORES)), trace=False)
    norm_w = np.asarray(inputs["norm_w"]).astype(np.float32)
    y0 = res.results[0]["y"]
    y1 = res.results[TP]["y"]
    y = np.concatenate([y0, y1], 0) * norm_w[None, :]
    return y.reshape(B, Q, H).astype(np.float32)
